# revision 1
# baseline (speedup 1.0000x reference)
"""GAT layer (nn_GATlayer) on 8 Trainium2 NeuronCores via Bass/Tile.

Strategy (edge parallelism over dst-sorted edges, per the sharding hint):
  - Host: sort edges by dst, shard dst nodes contiguously across 8 cores
    (6250 nodes/core), partition each core's nodes into blocks of 128.
    Each block's edges are split by src parity (even/odd node id) so the
    node table can be indexed with int16 dma_gather indices (idx = src>>1
    into a stride-2-rows view of the table); each parity group is padded
    to TL/TH tiles of 128 edges (global maxima, so one program serves all
    cores). Host also casts the node table to bf16 (hb): a gathered edge
    row is 512B instead of 1KB.
  - Device, per chunk of CB=4 blocks: TWO dma_gather instructions (even +
    odd parity) fetch all the chunk's h[src] rows in one SWDGE descgen
    pass each (~1us fixed + 0.34ns/edge) — v5 issued one indirect DMA per
    128 edges, each costing ~1us of Pool-serialized descgen, which
    dominated the whole kernel.
  - Device, per block:
      * selection mask S_T[e, t*128+n] = (dst_local[e,t] == n) via DVE
        compare of host-prepped dst arrays against an iota constant
      * edge logit a = (slab_slot0 . w1) + b[dst]; the dst-side per-node
        dot b[n] = h_block[n,0:D] . w2 is broadcast to edges by
        transposing b on the PE and selecting through S_T; leaky_relu;
        w = exp(a) (no softmax max-shift: logits are O(5) and softmax is
        shift-invariant, fp32 exp handles it exactly)
      * PE matmuls S_w^T @ slab accumulate messages in PSUM across tiles;
        paired 1-col matmuls against a ones column accumulate the
        denominator s (the gathered slab must stay contiguous for
        dma_gather, so the ones column lives outside it); out = msg/s;
        empty nodes (s==0) keep h.
  - Padding edges gather node 0/1 (benign real data) and carry
    dst_local=SENT so their mask columns are all-zero: they contribute
    exactly nothing.
"""
import math

import numpy as np

import concourse.bacc as bacc
import concourse.bass as bass
import concourse.tile as tile
from concourse import mybir
from concourse.bass import AP
from concourse.bass_utils import run_bass_kernel_spmd

import bass_rust

# problem dims (overridable for small-scale sim tests)
CFG = dict(N=50000, S=2, D=128, n_cores=8)
SLOPE = 0.98
P = 128
SENT = 200.0
CB = 4  # blocks per gather chunk

_f32 = mybir.dt.float32
_i16 = mybir.dt.int16
_bf16 = mybir.dt.bfloat16


def _dims():
    n, s, d, ncores = CFG["N"], CFG["S"], CFG["D"], CFG["n_cores"]
    return n, s, d, s * d, ncores, n // ncores, math.ceil(n // ncores / P)


def _split_sync_waits(nc, max_waits=1):
    """walrus in this container allows only 1 sync wait per instruction;
    move overflow waits onto preceding nops on the same engine."""
    for f in nc.m.functions:
        for bb in f.blocks:
            insts = bb.instructions
            i = 0
            while i < len(insts):
                ins = insts[i]
                si = ins.sync_info
                if si is not None and len(si.on_wait) > max_waits:
                    waits = list(si.on_wait)
                    si.on_wait = waits[-max_waits:]
                    overflow = waits[:-max_waits]
                    eng = nc.engines[ins.engine]
                    nops = []
                    for j in range(0, len(overflow), max_waits):
                        nop = eng.nop(hint="split_wait", nofuse=True)
                        nop.ins.sync_info = bass_rust.SyncInfo(
                            on_wait=overflow[j : j + max_waits], on_update=[]
                        )
                        nops.append(nop.ins)
                    for f2 in nc.m.functions:
                        for bb2 in f2.blocks:
                            bb2.instructions[:] = [
                                x for x in bb2.instructions if x not in nops
                            ]
                    for k, nop_ins in enumerate(nops):
                        insts.insert(i + k, nop_ins)
                    i += len(nops)
                i += 1


def host_prep(src, dst):
    """Sort edges by dst; build per-core parity-split index arrays.

    Per block b the edges are split into even-src and odd-src groups,
    each padded to TL/TH tiles of 128 (global maxima). Edge slots:
      even tile t, slot p  ->  dst_cm col b*(TL+TH)+t,        partition p
      odd  tile t, slot p  ->  dst_cm col b*(TL+TH)+TL+t,     partition p
    dma_gather index order k = t*128+p matches (p, t) slab placement.
    idx arrays are int16, wrapped [16c + k%16, k//16] for the 8 Q7 cores.
    """
    n, s, d, sd, ncores, npc, nb = _dims()
    order = np.argsort(dst, kind="stable")
    s_src = np.ascontiguousarray(src[order]).astype(np.int64)
    s_dst = np.ascontiguousarray(dst[order]).astype(np.int64)

    groups = []  # (core, block) -> (lo_idx_array, hi_idx_array, dstl, dsth)
    for c in range(ncores):
        for b in range(nb):
            lo = c * npc + b * P
            hi = min(lo + P, (c + 1) * npc)
            e0 = np.searchsorted(s_dst, lo, side="left")
            e1 = np.searchsorted(s_dst, hi, side="left")
            es = s_src[e0:e1]
            ed = (s_dst[e0:e1] - lo).astype(np.float32)
            even = (es % 2) == 0
            groups.append((es[even] >> 1, (es[~even] - 1) >> 1,
                           ed[even], ed[~even]))
    TL = max(1, max(math.ceil(len(g[0]) / P) for g in groups))
    TH = max(1, max(math.ceil(len(g[1]) / P) for g in groups))

    TT = TL + TH
    per_core = []
    for c in range(ncores):
        dst_cm = np.full((P, nb * TT), SENT, np.float32)  # also used as fp32 mask_start
        kl = np.zeros(nb * TL * P, np.int16)
        kh = np.zeros(nb * TH * P, np.int16)
        for b in range(nb):
            gl, gh, dl, dh = groups[c * nb + b]
            kl[b * TL * P : b * TL * P + len(gl)] = gl
            kh[b * TH * P : b * TH * P + len(gh)] = gh
            dbuf = np.full(TL * P, SENT, np.float32)
            dbuf[: len(dl)] = dl
            dst_cm[:, b * TT : b * TT + TL] = dbuf.reshape(TL, P).T
            dbuf = np.full(TH * P, SENT, np.float32)
            dbuf[: len(dh)] = dh
            dst_cm[:, b * TT + TL : (b + 1) * TT] = dbuf.reshape(TH, P).T
        # 16-partition wrap, replicated into all 8 Q7 core windows
        idx_lo = np.zeros((P, nb * TL * P // 16), np.int16)
        idx_hi = np.zeros((P, nb * TH * P // 16), np.int16)
        wl = kl.reshape(-1, 16).T  # [16, cols]
        wh = kh.reshape(-1, 16).T
        for q in range(8):
            idx_lo[16 * q : 16 * q + 16, :] = wl
            idx_hi[16 * q : 16 * q + 16, :] = wh
        per_core.append(
            {"idx_lo": idx_lo, "idx_hi": idx_hi, "dst_cm": dst_cm}
        )
    return (TL, TH), per_core


def build_program(TLH, reps=1, mode="full"):
    """Build the SPMD Bass program (v7: chunked dma_gather, parity split)."""
    import ml_dtypes  # noqa: F401
    TL, TH = TLH
    n, s, d, sd, ncores, npc, nb = _dims()
    TT = TL + TH
    nhalf = n // 2
    nc = bacc.Bacc("TRN2", target_bir_lowering=False, debug=False,
                   num_devices=ncores)
    hb_d = nc.dram_tensor("hb", [n, sd], _bf16, kind="ExternalInput").ap()
    wb_d = nc.dram_tensor("wb", [P, 2 * d], _f32, kind="ExternalInput").ap()
    w1b_d = nc.dram_tensor("w1b", [P, d], _bf16, kind="ExternalInput").ap()
    irb_d = nc.dram_tensor("irb", [P, P], _bf16, kind="ExternalInput").ap()
    id_d = nc.dram_tensor("id128", [P, P], _f32, kind="ExternalInput").ap()
    or_d = nc.dram_tensor("ones_row", [1, P], _f32, kind="ExternalInput").ap()
    il_d = nc.dram_tensor("idx_lo", [P, nb * TL * P // 16], _i16,
                          kind="ExternalInput").ap()
    ih_d = nc.dram_tensor("idx_hi", [P, nb * TH * P // 16], _i16,
                          kind="ExternalInput").ap()
    dstcm_d = nc.dram_tensor("dst_cm", [P, nb * TT], _bf16,
                             kind="ExternalInput").ap()
    hcm_d = nc.dram_tensor("hblk_cm", [P, nb * sd], _f32,
                           kind="ExternalInput").ap()
    y_d = nc.dram_tensor("y", [npc, sd], _f32, kind="ExternalOutput").ap()

    # parity views of the node table: row stride 2*sd, 256-elem rows
    hb_even = AP(hb_d.tensor, 0, [(2 * sd, nhalf), (1, sd)])
    hb_odd = AP(hb_d.tensor, sd, [(2 * sd, nhalf), (1, sd)])

    nchunks = math.ceil(nb / CB)
    CTMAX = CB * TT

    with tile.TileContext(nc) as tc:
        with (
            tc.tile_pool(name="const", bufs=1) as cpool,
            tc.tile_pool(name="psum", bufs=2, space="PSUM") as ppool,
        ):
            idx_lo = cpool.tile([P, nb * TL * P // 16], _i16)
            nc.sync.dma_start(out=idx_lo[:], in_=il_d[:])
            idx_hi = cpool.tile([P, nb * TH * P // 16], _i16)
            nc.sync.dma_start(out=idx_hi[:], in_=ih_d[:])
            dst_cm = cpool.tile([P, nb * TT], _bf16)
            nc.sync.dma_start(out=dst_cm[:], in_=dstcm_d[:])
            wb = cpool.tile([P, 2 * d], _f32)
            nc.sync.dma_start(out=wb[:], in_=wb_d[:])
            w1b = cpool.tile([P, d], _bf16)
            nc.sync.dma_start(out=w1b[:], in_=w1b_d[:])
            irb = cpool.tile([P, P], _bf16)
            nc.sync.dma_start(out=irb[:], in_=irb_d[:])
            id128 = cpool.tile([P, P], _f32)
            nc.sync.dma_start(out=id128[:], in_=id_d[:])
            ones_row = cpool.tile([1, P], _f32)
            nc.sync.dma_start(out=ones_row[:], in_=or_d[:])
            hcm = cpool.tile([P, nb * sd], _f32)
            nc.sync.dma_start(out=hcm[:], in_=hcm_d[:])
            b_cols = cpool.tile([P, nb], _f32)
            ones_col = cpool.tile([P, 1], _bf16)
            nc.vector.memset(ones_col[:], 1.0)

            # phase 0: b[n] = h[n, 0:d] . w2 for all this core's nodes
            with tc.tile_pool(name="init", bufs=1) as ipool:
                bmul_all = ipool.tile([P, nb * d], _f32)
                hcm_ap = hcm[:]
                h0_v = AP(hcm_ap.tensor, hcm_ap.offset,
                          [hcm_ap.ap[0], (sd, nb), (1, d)])
                w2_sl = wb[:, d : 2 * d]
                w2_rep = AP(w2_sl.tensor, w2_sl.offset,
                            [w2_sl.ap[0], (0, nb), (1, d)])
                nc.vector.tensor_tensor(
                    out=bmul_all[:], in0=h0_v, in1=w2_rep,
                    op=mybir.AluOpType.mult,
                )
                bm = bmul_all[:]
                bm_v = AP(bm.tensor, bm.offset, [bm.ap[0], (d, nb), (1, d)])
                nc.vector.tensor_reduce(
                    out=b_cols[:], in_=bm_v, axis=mybir.AxisListType.X,
                    op=mybir.AluOpType.add,
                )
            with tc.tile_pool(name="work", bufs=2) as wpool:
                def stage_gather(ch):
                    """two dma_gathers for chunk ch; returns chunk slab."""
                    b0 = ch * CB
                    bcnt = min(CB, nb - b0)
                    slab = wpool.tile([P, CTMAX, sd], _bf16, tag="slab")
                    nlo = bcnt * TL * P
                    nc.gpsimd.dma_gather(
                        out_ap=slab[:, 0 : bcnt * TL, 0:sd],
                        in_ap=hb_even,
                        idxs_ap=idx_lo[:, b0 * TL * 8 : (b0 + bcnt) * TL * 8],
                        num_idxs=nlo,
                        num_idxs_reg=nlo,
                        elem_size=sd,
                        elem_step=2 * sd,
                        single_packet=False,
                    )
                    nhi = bcnt * TH * P
                    nc.gpsimd.dma_gather(
                        out_ap=slab[:, bcnt * TL : bcnt * TT, 0:sd],
                        in_ap=hb_odd,
                        idxs_ap=idx_hi[:, b0 * TH * 8 : (b0 + bcnt) * TH * 8],
                        num_idxs=nhi,
                        num_idxs_reg=nhi,
                        elem_size=sd,
                        elem_step=2 * sd,
                        single_packet=False,
                    )
                    return slab

                def slab_pos(ch, b, t):
                    """slab tile index of block b's tile t within chunk ch."""
                    b0 = ch * CB
                    bcnt = min(CB, nb - b0)
                    i = b - b0
                    if t < TL:
                        return i * TL + t
                    return bcnt * TL + i * TH + (t - TL)

                def stage_a(ch, b, slab):
                    """masks + logits + S_w for block b."""
                    g0 = b * TT
                    # S_T[e, t*P+n] = (dst_cm[e, g0+t] == n)
                    s_t = wpool.tile([P, TT * P], _bf16, tag="s_t")
                    dcm_sl = dst_cm[:, g0 : g0 + TT]
                    dcm_rep = AP(dcm_sl.tensor, dcm_sl.offset,
                                 dcm_sl.ap + [(0, P)])
                    ir_sl = irb[:, 0:P]
                    ir_rep = AP(ir_sl.tensor, ir_sl.offset,
                                [ir_sl.ap[0], (0, TT), (1, P)])
                    nc.vector.tensor_tensor(
                        out=s_t[:], in0=dcm_rep, in1=ir_rep,
                        op=mybir.AluOpType.is_equal,
                    )

                    # broadcast this block's b value row to all partitions
                    bt_ps = ppool.tile([1, P], _f32, tag="bt")
                    nc.tensor.matmul(out=bt_ps[:],
                                     lhsT=b_cols[:, b : b + 1],
                                     rhs=id128[:], start=True, stop=True)
                    b_row = wpool.tile([1, P], _f32, tag="b_row")
                    nc.vector.tensor_copy(out=b_row[:], in_=bt_ps[:])
                    bb_ps = ppool.tile([P, P], _f32, tag="bb")
                    nc.tensor.matmul(out=bb_ps[:], lhsT=ones_row[:],
                                     rhs=b_row[:], start=True, stop=True)
                    b_bc = wpool.tile([P, P], _bf16, tag="b_bc")
                    nc.vector.tensor_copy(out=b_bc[:], in_=bb_ps[:])

                    # select b[dst_e]: bsl = S_T * b_bc, b_mat = sum_n
                    bsl = wpool.tile([P, TT * P], _bf16, tag="bsl")
                    bb_sl = b_bc[:, 0:P]
                    bb_rep = AP(bb_sl.tensor, bb_sl.offset,
                                [bb_sl.ap[0], (0, TT), (1, P)])
                    nc.vector.tensor_tensor(
                        out=bsl[:], in0=s_t[:], in1=bb_rep,
                        op=mybir.AluOpType.mult,
                    )
                    b_mat = wpool.tile([P, TT], _f32, tag="b_mat")
                    bsl_ap = bsl[:]
                    bsl_v = AP(bsl_ap.tensor, bsl_ap.offset,
                               [bsl_ap.ap[0], (P, TT), (1, P)])
                    nc.vector.tensor_reduce(
                        out=b_mat[:], in_=bsl_v, axis=mybir.AxisListType.X,
                        op=mybir.AluOpType.add,
                    )

                    # src-side dot over the chunk slab's tiles of this block
                    dmul = wpool.tile([P, TT * d], _bf16, tag="dmul")
                    w1_sl = w1b[:, 0:d]
                    dot = wpool.tile([P, TT], _f32, tag="dot")
                    sl_ap = slab[:]
                    p_lo = slab_pos(ch, b, 0)
                    lo_v = AP(sl_ap.tensor, sl_ap.offset + p_lo * sd,
                              [sl_ap.ap[0], (sd, TL), (1, d)])
                    p_hi = slab_pos(ch, b, TL)
                    hi_v = AP(sl_ap.tensor, sl_ap.offset + p_hi * sd,
                              [sl_ap.ap[0], (sd, TH), (1, d)])
                    nc.vector.tensor_tensor(
                        out=dmul[:, 0 : TL * d], in0=lo_v,
                        in1=AP(w1_sl.tensor, w1_sl.offset,
                               [w1_sl.ap[0], (0, TL), (1, d)]),
                        op=mybir.AluOpType.mult,
                    )
                    nc.vector.tensor_tensor(
                        out=dmul[:, TL * d : TT * d], in0=hi_v,
                        in1=AP(w1_sl.tensor, w1_sl.offset,
                               [w1_sl.ap[0], (0, TH), (1, d)]),
                        op=mybir.AluOpType.mult,
                    )
                    dm = dmul[:]
                    dmul_v = AP(dm.tensor, dm.offset,
                                [dm.ap[0], (d, TT), (1, d)])
                    nc.vector.tensor_reduce(
                        out=dot[:], in_=dmul_v, axis=mybir.AxisListType.X,
                        op=mybir.AluOpType.add,
                    )

                    # a = dot + b ; leaky ; w = exp(a) (ACT, bf16 out)
                    a_mat = wpool.tile([P, TT], _f32, tag="a_mat")
                    nc.vector.tensor_tensor(
                        out=a_mat[:], in0=dot[:], in1=b_mat[:],
                        op=mybir.AluOpType.add,
                    )
                    a_sc = wpool.tile([P, TT], _f32, tag="a_sc")
                    nc.vector.tensor_scalar_mul(a_sc[:], a_mat[:], SLOPE)
                    nc.vector.tensor_tensor(
                        out=a_mat[:], in0=a_mat[:], in1=a_sc[:],
                        op=mybir.AluOpType.max,
                    )
                    w_mat = wpool.tile([P, TT], _bf16, tag="w_mat")
                    nc.scalar.activation(
                        out=w_mat[:], in_=a_mat[:],
                        func=mybir.ActivationFunctionType.Exp,
                    )

                    # S_w = S_T * w
                    wm_sl = w_mat[:, 0:TT]
                    wm_rep = AP(wm_sl.tensor, wm_sl.offset,
                                wm_sl.ap + [(0, P)])
                    nc.vector.tensor_tensor(
                        out=s_t[:], in0=s_t[:], in1=wm_rep,
                        op=mybir.AluOpType.mult,
                    )
                    return s_t

                def stage_mm(ch, b, slab, s_t):
                    """PE accumulation for block b; returns (acc, den)."""
                    acc = ppool.tile([P, sd], _f32, tag="acc")
                    den = ppool.tile([P, 1], _f32, tag="den")
                    for t in range(TT):
                        pos = slab_pos(ch, b, t)
                        lhsT = s_t[:, t * P : (t + 1) * P]
                        nc.tensor.matmul(
                            out=acc[:],
                            lhsT=lhsT,
                            rhs=slab[:, pos, 0:sd],
                            start=(t == 0), stop=(t == TT - 1),
                        )
                        nc.tensor.matmul(
                            out=den[:],
                            lhsT=lhsT,
                            rhs=ones_col[:, 0:1],
                            start=(t == 0), stop=(t == TT - 1),
                        )
                    return acc, den

                def stage_fin(b, acc, den):
                    """normalization + blend + store for block b."""
                    node_lo = b * P
                    nrows = min(P, npc - node_lo)
                    h_block = hcm[:, b * sd : (b + 1) * sd]
                    s_col = den[:, 0:1]

                    eq0 = wpool.tile([P, 1], _f32, tag="eq0")
                    nc.vector.tensor_scalar(
                        out=eq0[:], in0=s_col, scalar1=0.0,
                        scalar2=None, op0=mybir.AluOpType.is_equal,
                    )
                    s_safe = wpool.tile([P, 1], _f32, tag="s_safe")
                    nc.vector.tensor_tensor(
                        out=s_safe[:], in0=s_col, in1=eq0[:],
                        op=mybir.AluOpType.add,
                    )
                    rec = wpool.tile([P, 1], _f32, tag="rec")
                    nc.vector.reciprocal(out=rec[:], in_=s_safe[:])
                    out_sb = wpool.tile([P, sd], _f32, tag="out_sb")
                    nc.vector.tensor_scalar(
                        out=out_sb[:], in0=acc[:], scalar1=rec[:, 0:1],
                        scalar2=None, op0=mybir.AluOpType.mult,
                    )
                    hmask = wpool.tile([P, sd], _f32, tag="hmask")
                    nc.vector.tensor_scalar(
                        out=hmask[:], in0=h_block, scalar1=eq0[:, 0:1],
                        scalar2=None, op0=mybir.AluOpType.mult,
                    )
                    nc.vector.tensor_tensor(
                        out=out_sb[:], in0=out_sb[:], in1=hmask[:],
                        op=mybir.AluOpType.add,
                    )
                    nc.sync.dma_start(
                        out=y_d[:][node_lo : node_lo + nrows, :],
                        in_=out_sb[:nrows, :],
                    )

                def fake_gather(ch):
                    slab = wpool.tile([P, CTMAX, sd], _bf16, tag="slab")
                    nc.vector.memset(slab[:, 0:1, 0:sd], 1.0)
                    return slab

                def process_chunk(ch, slab):
                    b0, b1 = ch * CB, min((ch + 1) * CB, nb)
                    pend_a = None   # (b, s_t)
                    pend_mm = None  # (b, acc, den)
                    for b in range(b0, b1):
                        s_t = stage_a(ch, b, slab)
                        if pend_mm is not None:
                            stage_fin(pend_mm[0], pend_mm[1], pend_mm[2])
                            pend_mm = None
                        if pend_a is not None:
                            acc, den = stage_mm(ch, pend_a[0], slab,
                                                pend_a[1])
                            pend_mm = (pend_a[0], acc, den)
                        pend_a = (b, s_t)
                    acc, den = stage_mm(ch, pend_a[0], slab, pend_a[1])
                    if pend_mm is not None:
                        stage_fin(pend_mm[0], pend_mm[1], pend_mm[2])
                    stage_fin(pend_a[0], acc, den)

                for rep in range(reps):
                    prev = None  # (ch, slab)
                    for ch in range(nchunks):
                        if mode == "compute":
                            slab = fake_gather(ch)
                        else:
                            slab = stage_gather(ch)
                        if mode == "gather":
                            continue
                        if prev is not None:
                            process_chunk(prev[0], prev[1])
                        prev = (ch, slab)
                    if mode == "gather":
                        continue
                    process_chunk(prev[0], prev[1])

    nc.compile()
    _split_sync_waits(nc, max_waits=1)
    return nc


_cache = {}


def make_in_maps(h_features, w_att, per_core):
    import ml_dtypes
    bf16 = np.dtype(ml_dtypes.bfloat16)
    n, s, d, sd, ncores, npc, nb = _dims()
    TT = per_core[0]["dst_cm"].shape[1] // nb
    h2 = np.ascontiguousarray(h_features.reshape(n, sd), dtype=np.float32)
    hb = h2.astype(bf16)
    w_flat = np.ascontiguousarray(w_att.reshape(1, 2 * d), dtype=np.float32)
    wb = np.repeat(w_flat, P, axis=0)
    w1b = np.ascontiguousarray(wb[:, 0:d]).astype(bf16)
    irb = np.repeat(np.arange(P, dtype=np.float32).reshape(1, P), P,
                    axis=0).astype(bf16)
    id128 = np.eye(P, dtype=np.float32)
    ones_row = np.ones((1, P), np.float32)
    hcm_list = []
    for c in range(ncores):
        pad_rows = nb * P
        hp = np.zeros((pad_rows, sd), np.float32)
        hp[:npc] = h2[c * npc : (c + 1) * npc]
        hcm_list.append(
            np.ascontiguousarray(
                hp.reshape(nb, P, sd).transpose(1, 0, 2).reshape(P, nb * sd)
            )
        )
    in_maps = []
    for c in range(ncores):
        dst_cm = per_core[c]["dst_cm"]
        in_maps.append(
            {
                "hb": hb,
                "wb": wb,
                "w1b": w1b,
                "irb": irb,
                "id128": id128,
                "ones_row": ones_row,
                "idx_lo": per_core[c]["idx_lo"],
                "idx_hi": per_core[c]["idx_hi"],
                "dst_cm": dst_cm.astype(bf16),
                "hblk_cm": hcm_list[c],
            }
        )
    return in_maps


def kernel(h_features, src, dst, w_att):
    n, s, d, sd, ncores, npc, nb = _dims()
    h_features = np.ascontiguousarray(h_features, dtype=np.float32)
    src = np.ascontiguousarray(src, dtype=np.int32)
    dst = np.ascontiguousarray(dst, dtype=np.int32)
    w_att = np.ascontiguousarray(w_att, dtype=np.float32)

    TLH, per_core = host_prep(src, dst)
    if TLH not in _cache:
        _cache[TLH] = build_program(TLH)
    nc = _cache[TLH]

    in_maps = make_in_maps(h_features, w_att, per_core)
    res = run_bass_kernel_spmd(nc, in_maps, list(range(ncores)))
    out = np.concatenate([res.results[c]["y"] for c in range(ncores)], axis=0)
    return out.reshape(n, s, d).astype(np.float32)



# revision 3
# speedup vs baseline: 24.0845x; 24.0845x over previous
"""GAT layer (nn_GATlayer) on 8 Trainium2 NeuronCores via Bass/Tile.

v8: degree-sorted DIAGONAL edge layout (dst node == SBUF partition).

  - Host: rank all nodes by in-degree (desc). Global block g of 128
    consecutive ranks -> (core c = g%8, block b = g//8), so the 8 cores'
    b-th blocks hold adjacent degree ranks and share tile counts.
    Within a block, node p's incoming edges occupy slots (p, t): the
    whole segment softmax + scatter becomes per-partition arithmetic:
      * b_dst is a per-partition scalar (tensor_scalar bias) - no mask
        select needed.
      * the scatter matmul's lhsT is diag(w) - built by one 4x-mode
        tensor_scalar per tile (id128 * w_col), no is_equal masks.
      * the denominator is a free-dim reduce (scalar_tensor_tensor
        accum_out) - no ones-column matmuls.
  - int16 gather indices cover only 32768 rows, so each node's edge
    list is split into a "lo" prefix (src row < 32768, gathered from
    table view rows [0, 32768)) and a "hi" suffix (src >= 17232,
    gathered from view rows [17232, 50000)). Nodes 17232..32767 are
    reachable from both views, which lets the host balance each node's
    split around deg/2 with tiny padding. Padding slots gather row 0
    of the view (benign real data) and carry mask 0.
  - Per-edge src logit c = h[src,0,:]@w1 is computed from the gathered
    slab: bf16 multiply (2x DVE) + one bf16 pair-fold + f32 reduce.
    Then a = c + b_dst (tensor_scalar), leaky via scalar_tensor_tensor
    max, exp on ACT, w*mask + denominator in one scalar_tensor_tensor.
  - out = acc/den via ACT copy with per-partition scale; empty/ragged
    nodes (den==0 -> junk) are overwritten host-side with h (exact).
"""
import math

import numpy as np

import concourse.bacc as bacc
import concourse.bass as bass
import concourse.tile as tile
from concourse import mybir
from concourse.bass import AP
from concourse.bass_utils import run_bass_kernel_spmd

import bass_rust

# problem dims
CFG = dict(N=50000, S=2, D=128, n_cores=8)
SLOPE = 0.98
P = 128
LO_ROWS = 32768          # lo view: table rows [0, 32768)
HI_BASE = 17232          # hi view: table rows [17232, 50000)
TCH = 80                 # max slab tiles per gather chunk

_f32 = mybir.dt.float32
_i16 = mybir.dt.int16
_bf16 = mybir.dt.bfloat16


def _dims():
    n, s, d, ncores = CFG["N"], CFG["S"], CFG["D"], CFG["n_cores"]
    nbg = math.ceil(n / P / ncores) * ncores      # global blocks (392)
    nbu = nbg // ncores                           # blocks per core (49)
    return n, s, d, s * d, ncores, nbg, nbu


def _split_sync_waits(nc, max_waits=1):
    """walrus in this container allows only 1 sync wait per instruction;
    move overflow waits onto preceding nops on the same engine."""
    for f in nc.m.functions:
        for bb in f.blocks:
            insts = bb.instructions
            i = 0
            while i < len(insts):
                ins = insts[i]
                si = ins.sync_info
                if si is not None and len(si.on_wait) > max_waits:
                    waits = list(si.on_wait)
                    si.on_wait = waits[-max_waits:]
                    overflow = waits[:-max_waits]
                    eng = nc.engines[ins.engine]
                    nops = []
                    for j in range(0, len(overflow), max_waits):
                        nop = eng.nop(hint="split_wait", nofuse=True)
                        nop.ins.sync_info = bass_rust.SyncInfo(
                            on_wait=overflow[j : j + max_waits], on_update=[]
                        )
                        nops.append(nop.ins)
                    for f2 in nc.m.functions:
                        for bb2 in f2.blocks:
                            bb2.instructions[:] = [
                                x for x in bb2.instructions if x not in nops
                            ]
                    for k, nop_ins in enumerate(nops):
                        insts.insert(i + k, nop_ins)
                    i += len(nops)
                i += 1


def _wrap16(flat):
    """int16 slot array -> [128, len/16] 16-partition wrap replicated x8."""
    w = np.ascontiguousarray(flat.reshape(-1, 16).T)  # [16, cols]
    out = np.zeros((P, w.shape[1]), np.int16)
    for q in range(8):
        out[16 * q : 16 * q + 16, :] = w
    return out


def host_prep(src, dst):
    """Degree-sort nodes, build diagonal slot arrays + lo/hi split.

    Returns (sched, per_core):
      sched = (tuple(TL), tuple(TH), tuple(chunks)) - compile-time shape
      per_core[c] = dict(idx_lo, idx_hi, m) + aux keys (order, deg, nlow)
    """
    n, s, d, sd, ncores, nbg, nbu = _dims()
    src = src.astype(np.int64)
    dst = dst.astype(np.int64)
    deg = np.bincount(dst, minlength=n)
    # per-node feasible split point v = clip(deg/2, strict_lo, strict_lo+flex)
    # (strict_lo = #srcs reachable only from the lo view, flex = from both);
    # rank nodes by (deg desc, v desc) so each block is homogeneous in BOTH
    # the tile count and the lo/hi boundary -> minimal padding.
    slo_n = np.bincount(dst[src < HI_BASE], minlength=n)
    flex_n = np.bincount(dst[(src >= HI_BASE) & (src < LO_ROWS)], minlength=n)
    v_n = np.clip((deg + 1) // 2, slo_n, slo_n + flex_n)
    order = np.lexsort((-v_n, -deg)).astype(np.int64)  # rank -> node
    rank_of = np.empty(n, np.int64)
    rank_of[order] = np.arange(n)

    ngrid = nbg * P
    degr = np.zeros(ngrid, np.int64)
    degr[:n] = deg[order]

    # per-node lo/hi split
    er = rank_of[dst]
    eo = np.argsort(er, kind="stable")
    es = src[eo]
    ranks_e = er[eo]
    cls = np.full(len(es), 1, np.int64)           # 1 = flex
    cls[es < HI_BASE] = 0                          # strict lo
    cls[es >= LO_ROWS] = 2                         # strict hi
    eo2 = np.argsort(ranks_e * 4 + cls, kind="stable")
    es2 = es[eo2]
    ranks2 = ranks_e[eo2]
    nlow = np.zeros(ngrid, np.int64)
    nlow[:n] = v_n[order[:n]]

    # per-core-block tile counts (max over the 8 cores' b-th blocks)
    nl_g = nlow.reshape(nbg, P)
    nh_g = (degr - nlow).reshape(nbg, P)
    TL = np.zeros(nbu, np.int64)
    TH = np.zeros(nbu, np.int64)
    for b in range(nbu):
        TL[b] = nl_g[b * ncores : (b + 1) * ncores].max()
        TH[b] = nh_g[b * ncores : (b + 1) * ncores].max()
    LOFF = np.zeros(nbu + 1, np.int64)
    LOFF[1:] = np.cumsum(TL)
    HOFF = np.zeros(nbu + 1, np.int64)
    HOFF[1:] = np.cumsum(TH)
    MOFF = np.zeros(nbu + 1, np.int64)
    MOFF[1:] = np.cumsum(TL + TH)

    # chunk schedule: consecutive blocks, sum(TL+TH) <= TCH
    chunks = []
    b0 = 0
    while b0 < nbu:
        b1 = b0 + 1
        while b1 < nbu and (LOFF[b1 + 1] - LOFF[b0]) + (
            HOFF[b1 + 1] - HOFF[b0]
        ) <= TCH:
            b1 += 1
        chunks.append((b0, b1))
        b0 = b1

    # per-edge slot assignment (vectorized)
    starts = np.zeros(ngrid + 1, np.int64)
    starts[1:] = np.cumsum(degr)
    epos = np.arange(len(es2)) - starts[ranks2]    # position within node
    g = ranks2 // P
    p = ranks2 % P
    ecore = g % ncores
    eb = g // ncores
    is_lo = epos < nlow[ranks2]
    t_lo = epos
    t_hi = epos - nlow[ranks2]

    per_core = []
    for c in range(ncores):
        kl = np.zeros(LOFF[nbu] * P, np.int16)
        kh = np.zeros(HOFF[nbu] * P, np.int16)
        m = np.zeros((P, MOFF[nbu]), np.float32)
        sel = ecore == c
        sl = sel & is_lo
        sh = sel & ~is_lo
        slot_l = (LOFF[eb[sl]] + t_lo[sl]) * P + p[sl]
        kl[slot_l] = es2[sl].astype(np.int16)
        m[p[sl], MOFF[eb[sl]] + t_lo[sl]] = 1.0
        slot_h = (HOFF[eb[sh]] + t_hi[sh]) * P + p[sh]
        kh[slot_h] = (es2[sh] - HI_BASE).astype(np.int16)
        m[p[sh], MOFF[eb[sh]] + TL[eb[sh]] + t_hi[sh]] = 1.0
        per_core.append(
            {
                "idx_lo": _wrap16(kl),
                "idx_hi": _wrap16(kh),
                "m": m,
            }
        )
    per_core[0]["_order"] = order
    per_core[0]["_deg"] = deg
    sched = (tuple(int(x) for x in TL), tuple(int(x) for x in TH),
             tuple(chunks))
    return sched, per_core


def build_program(sched, reps=1, mode="full"):
    """Build the SPMD Bass program (v8 diagonal)."""
    import ml_dtypes  # noqa: F401
    TL, TH, chunks = sched
    n, s, d, sd, ncores, nbg, nbu = _dims()
    TLa = np.array(TL)
    THa = np.array(TH)
    LOFF = np.zeros(nbu + 1, np.int64)
    LOFF[1:] = np.cumsum(TLa)
    HOFF = np.zeros(nbu + 1, np.int64)
    HOFF[1:] = np.cumsum(THa)
    MOFF = np.zeros(nbu + 1, np.int64)
    MOFF[1:] = np.cumsum(TLa + THa)
    Tmax = int((TLa + THa).max())
    sumT = int(MOFF[nbu])

    nc = bacc.Bacc("TRN2", target_bir_lowering=False, debug=False,
                   num_devices=ncores)
    hb_d = nc.dram_tensor("hb", [n, sd], _bf16, kind="ExternalInput").ap()
    w1_d = nc.dram_tensor("w1b", [P, d], _bf16, kind="ExternalInput").ap()
    w2_d = nc.dram_tensor("w2b", [P, d], _bf16, kind="ExternalInput").ap()
    id_d = nc.dram_tensor("id128", [P, P], _bf16, kind="ExternalInput").ap()
    il_d = nc.dram_tensor("idx_lo", [P, int(LOFF[nbu]) * P // 16], _i16,
                          kind="ExternalInput").ap()
    ih_d = nc.dram_tensor("idx_hi", [P, int(HOFF[nbu]) * P // 16], _i16,
                          kind="ExternalInput").ap()
    m_d = nc.dram_tensor("m_cm", [P, sumT], _bf16, kind="ExternalInput").ap()
    hcm_d = nc.dram_tensor("hcm0", [P, nbu * d], _bf16,
                           kind="ExternalInput").ap()
    y_d = nc.dram_tensor("y", [nbu * P, sd], _f32, kind="ExternalOutput").ap()

    hb_lo = AP(hb_d.tensor, 0, [(sd, LO_ROWS), (1, sd)])
    hb_hi = AP(hb_d.tensor, HI_BASE * sd, [(sd, n - HI_BASE), (1, sd)])

    with tile.TileContext(nc) as tc:
        with (
            tc.tile_pool(name="const", bufs=1) as cpool,
            tc.tile_pool(name="psum", bufs=2, space="PSUM") as ppool,
        ):
            idx_lo = cpool.tile([P, int(LOFF[nbu]) * P // 16], _i16)
            nc.sync.dma_start(out=idx_lo[:], in_=il_d[:])
            idx_hi = cpool.tile([P, int(HOFF[nbu]) * P // 16], _i16)
            nc.sync.dma_start(out=idx_hi[:], in_=ih_d[:])
            m_cm = cpool.tile([P, sumT], _bf16)
            nc.sync.dma_start(out=m_cm[:], in_=m_d[:])
            w1b = cpool.tile([P, d], _bf16)
            nc.sync.dma_start(out=w1b[:], in_=w1_d[:])
            w2b = cpool.tile([P, d], _bf16)
            nc.sync.dma_start(out=w2b[:], in_=w2_d[:])
            id128 = cpool.tile([P, P], _bf16)
            nc.sync.dma_start(out=id128[:], in_=id_d[:])
            hcm0 = cpool.tile([P, nbu * d], _bf16)
            nc.sync.dma_start(out=hcm0[:], in_=hcm_d[:])
            b_cols = cpool.tile([P, nbu], _f32)

            # phase 0: b[dst] = h0 . w2 for this core's (permuted) nodes
            with tc.tile_pool(name="init", bufs=1) as ipool:
                bmul = ipool.tile([P, nbu * d], _bf16)
                hv = hcm0[:]
                w2_sl = w2b[:, 0:d]
                nc.vector.tensor_tensor(
                    out=bmul[:],
                    in0=AP(hv.tensor, hv.offset, [hv.ap[0], (d, nbu), (1, d)]),
                    in1=AP(w2_sl.tensor, w2_sl.offset,
                           [w2_sl.ap[0], (0, nbu), (1, d)]),
                    op=mybir.AluOpType.mult,
                )
                bm = bmul[:]
                nc.vector.tensor_reduce(
                    out=b_cols[:],
                    in_=AP(bm.tensor, bm.offset, [bm.ap[0], (d, nbu), (1, d)]),
                    axis=mybir.AxisListType.X,
                    op=mybir.AluOpType.add,
                )

            with tc.tile_pool(name="work", bufs=2) as wpool:
                def stage_gather(ci):
                    b0, b1 = chunks[ci]
                    cTL = int(LOFF[b1] - LOFF[b0])
                    cTH = int(HOFF[b1] - HOFF[b0])
                    slab = wpool.tile([P, TCH, sd], _bf16, tag="slab")
                    if cTL:
                        nlo = cTL * P
                        nc.gpsimd.dma_gather(
                            out_ap=slab[:, 0:cTL, 0:sd],
                            in_ap=hb_lo,
                            idxs_ap=idx_lo[:, int(LOFF[b0]) * 8 :
                                           int(LOFF[b1]) * 8],
                            num_idxs=nlo,
                            num_idxs_reg=nlo,
                            elem_size=sd,
                            elem_step=sd,
                            single_packet=False,
                        )
                    if cTH:
                        nhi = cTH * P
                        nc.gpsimd.dma_gather(
                            out_ap=slab[:, cTL : cTL + cTH, 0:sd],
                            in_ap=hb_hi,
                            idxs_ap=idx_hi[:, int(HOFF[b0]) * 8 :
                                           int(HOFF[b1]) * 8],
                            num_idxs=nhi,
                            num_idxs_reg=nhi,
                            elem_size=sd,
                            elem_step=sd,
                            single_packet=False,
                        )
                    return slab

                def fake_gather(ci):
                    slab = wpool.tile([P, TCH, sd], _bf16, tag="slab")
                    nc.vector.memset(slab[:, 0:1, 0:sd], 1.0)
                    return slab

                def slab_pos(ci, b, t):
                    b0, b1 = chunks[ci]
                    cTL = int(LOFF[b1] - LOFF[b0])
                    if t < TL[b]:
                        return int(LOFF[b] - LOFF[b0]) + t
                    return cTL + int(HOFF[b] - HOFF[b0]) + (t - TL[b])

                def stage_a(ci, b, slab):
                    """per-edge logits -> masked weights w_mask [P, T]."""
                    T = TL[b] + TH[b]
                    sl = slab[:]
                    # dmul = slab_slot0 * w1 (bf16, 2x) in lo/hi pieces
                    dmul = wpool.tile([P, Tmax * d], _bf16, tag="dmul")
                    w1_sl = w1b[:, 0:d]
                    if TL[b]:
                        p_lo = slab_pos(ci, b, 0)
                        nc.vector.tensor_tensor(
                            out=dmul[:, 0 : TL[b] * d],
                            in0=AP(sl.tensor, sl.offset + p_lo * sd,
                                   [sl.ap[0], (sd, TL[b]), (1, d)]),
                            in1=AP(w1_sl.tensor, w1_sl.offset,
                                   [w1_sl.ap[0], (0, TL[b]), (1, d)]),
                            op=mybir.AluOpType.mult,
                        )
                    if TH[b]:
                        p_hi = slab_pos(ci, b, TL[b])
                        nc.vector.tensor_tensor(
                            out=dmul[:, TL[b] * d : T * d],
                            in0=AP(sl.tensor, sl.offset + p_hi * sd,
                                   [sl.ap[0], (sd, TH[b]), (1, d)]),
                            in1=AP(w1_sl.tensor, w1_sl.offset,
                                   [w1_sl.ap[0], (0, TH[b]), (1, d)]),
                            op=mybir.AluOpType.mult,
                        )
                    # pair-fold 128 -> 64 in bf16 (2x), then f32 reduce
                    fold = wpool.tile([P, Tmax * (d // 2)], _bf16, tag="fold")
                    dm = dmul[:]
                    nc.vector.tensor_tensor(
                        out=fold[:, 0 : T * (d // 2)],
                        in0=AP(dm.tensor, dm.offset,
                               [dm.ap[0], (d, T), (1, d // 2)]),
                        in1=AP(dm.tensor, dm.offset + d // 2,
                               [dm.ap[0], (d, T), (1, d // 2)]),
                        op=mybir.AluOpType.add,
                    )
                    dot = wpool.tile([P, Tmax], _f32, tag="dot")
                    fo = fold[:]
                    nc.vector.tensor_reduce(
                        out=dot[:, 0:T],
                        in_=AP(fo.tensor, fo.offset,
                               [fo.ap[0], (d // 2, T), (1, d // 2)]),
                        axis=mybir.AxisListType.X,
                        op=mybir.AluOpType.add,
                    )
                    # a = dot + b_dst ; leaky = max(a, SLOPE*a)
                    a_mat = wpool.tile([P, Tmax], _f32, tag="a_mat")
                    nc.vector.tensor_scalar(
                        out=a_mat[:, 0:T], in0=dot[:, 0:T],
                        scalar1=b_cols[:, b : b + 1], scalar2=None,
                        op0=mybir.AluOpType.add,
                    )
                    lr = wpool.tile([P, Tmax], _f32, tag="lr")
                    nc.vector.scalar_tensor_tensor(
                        out=lr[:, 0:T], in0=a_mat[:, 0:T], scalar=SLOPE,
                        in1=a_mat[:, 0:T], op0=mybir.AluOpType.mult,
                        op1=mybir.AluOpType.max,
                    )
                    # w = exp(lr) on ACT
                    w_exp = wpool.tile([P, Tmax], _f32, tag="w_exp")
                    nc.scalar.activation(
                        out=w_exp[:, 0:T], in_=lr[:, 0:T],
                        func=mybir.ActivationFunctionType.Exp,
                    )
                    # w_mask = w * m ; den = sum(w_mask)
                    w_mask = wpool.tile([P, Tmax], _f32, tag="w_mask")
                    den = wpool.tile([P, 1], _f32, tag="den")
                    nc.vector.scalar_tensor_tensor(
                        out=w_mask[:, 0:T], in0=w_exp[:, 0:T], scalar=1.0,
                        in1=m_cm[:, int(MOFF[b]) : int(MOFF[b]) + T],
                        op0=mybir.AluOpType.mult, op1=mybir.AluOpType.mult,
                        accum_out=den[:, 0:1],
                    )
                    rec = wpool.tile([P, 1], _f32, tag="rec")
                    nc.vector.reciprocal(out=rec[:], in_=den[:, 0:1])
                    return w_mask, rec

                def stage_mm(ci, b, slab, w_mask):
                    """diag(w) matmuls accumulating the block's messages."""
                    T = TL[b] + TH[b]
                    acc = ppool.tile([P, sd], _f32, tag="acc")
                    diag = wpool.tile([P, Tmax * P], _bf16, tag="diag")
                    for t in range(T):
                        nc.vector.tensor_scalar(
                            out=diag[:, t * P : (t + 1) * P],
                            in0=id128[:, 0:P],
                            scalar1=w_mask[:, t : t + 1], scalar2=None,
                            op0=mybir.AluOpType.mult,
                        )
                        nc.tensor.matmul(
                            out=acc[:],
                            lhsT=diag[:, t * P : (t + 1) * P],
                            rhs=slab[:, slab_pos(ci, b, t), 0:sd],
                            start=(t == 0), stop=(t == T - 1),
                        )
                    return acc

                def stage_fin(b, acc, rec):
                    out_sb = wpool.tile([P, sd], _f32, tag="out_sb")
                    nc.scalar.activation(
                        out=out_sb[:], in_=acc[:],
                        func=mybir.ActivationFunctionType.Copy,
                        scale=rec[:, 0:1],
                    )
                    nc.sync.dma_start(
                        out=y_d[:][b * P : (b + 1) * P, :],
                        in_=out_sb[:],
                    )

                def process_chunk(ci, slab):
                    b0, b1 = chunks[ci]
                    pend_a = None   # (b, w_mask, rec)
                    pend_mm = None  # (b, acc, rec)
                    for b in range(b0, b1):
                        if TL[b] + TH[b] == 0:
                            continue
                        w_mask, rec = stage_a(ci, b, slab)
                        if pend_mm is not None:
                            stage_fin(pend_mm[0], pend_mm[1], pend_mm[2])
                            pend_mm = None
                        if pend_a is not None:
                            acc = stage_mm(ci, pend_a[0], slab, pend_a[1])
                            pend_mm = (pend_a[0], acc, pend_a[2])
                        pend_a = (b, w_mask, rec)
                    if pend_a is not None:
                        acc = stage_mm(ci, pend_a[0], slab, pend_a[1])
                        if pend_mm is not None:
                            stage_fin(pend_mm[0], pend_mm[1], pend_mm[2])
                        stage_fin(pend_a[0], acc, pend_a[2])
                    elif pend_mm is not None:
                        stage_fin(pend_mm[0], pend_mm[1], pend_mm[2])

                for rep in range(reps):
                    prev = None
                    for ci in range(len(chunks)):
                        if mode == "compute":
                            slab = fake_gather(ci)
                        else:
                            slab = stage_gather(ci)
                        if mode == "gather":
                            continue
                        if prev is not None:
                            process_chunk(prev[0], prev[1])
                        prev = (ci, slab)
                    if mode == "gather":
                        continue
                    process_chunk(prev[0], prev[1])

    nc.compile()
    _split_sync_waits(nc, max_waits=1)
    return nc


_cache = {}


def make_in_maps(h_features, w_att, per_core):
    import ml_dtypes
    bf16 = np.dtype(ml_dtypes.bfloat16)
    n, s, d, sd, ncores, nbg, nbu = _dims()
    order = per_core[0]["_order"]
    h2 = np.ascontiguousarray(h_features.reshape(n, sd), dtype=np.float32)
    hb = h2.astype(bf16)
    w_flat = np.ascontiguousarray(w_att.reshape(2 * d), dtype=np.float32)
    w1b = np.repeat(w_flat[None, 0:d], P, axis=0).astype(bf16)
    w2b = np.repeat(w_flat[None, d : 2 * d], P, axis=0).astype(bf16)
    id128 = np.eye(P, dtype=np.float32).astype(bf16)

    # hcm0[p, b*128:(b+1)*128] = h0 of node at rank (b*8+c)*128+p
    h0 = h2[:, 0:d]
    in_maps = []
    for c in range(ncores):
        hcm0 = np.zeros((P, nbu * d), np.float32)
        for b in range(nbu):
            ranks = (b * ncores + c) * P + np.arange(P)
            valid = ranks < n
            nodes = order[ranks[valid]]
            hcm0[valid, b * d : (b + 1) * d] = h0[nodes]
        in_maps.append(
            {
                "hb": hb,
                "w1b": w1b,
                "w2b": w2b,
                "id128": id128,
                "idx_lo": per_core[c]["idx_lo"],
                "idx_hi": per_core[c]["idx_hi"],
                "m_cm": per_core[c]["m"].astype(bf16),
                "hcm0": hcm0.astype(bf16),
            }
        )
    return in_maps


def assemble(y_list, h_features, per_core):
    """Scatter per-core y rows back to original node order; deg==0 and
    grid-pad rows come from h_features exactly."""
    n, s, d, sd, ncores, nbg, nbu = _dims()
    order = per_core[0]["_order"]
    deg = per_core[0]["_deg"]
    out = np.array(h_features.reshape(n, sd), dtype=np.float32, copy=True)
    for c in range(ncores):
        y = y_list[c]
        for b in range(nbu):
            ranks = (b * ncores + c) * P + np.arange(P)
            valid = ranks < n
            nodes = order[ranks[valid]]
            live = deg[nodes] > 0
            out[nodes[live]] = y[b * P : (b + 1) * P][valid][live]
    return out.reshape(n, s, d)


def kernel(h_features, src, dst, w_att):
    n, s, d, sd, ncores, nbg, nbu = _dims()
    h_features = np.ascontiguousarray(h_features, dtype=np.float32)
    src = np.ascontiguousarray(src, dtype=np.int32)
    dst = np.ascontiguousarray(dst, dtype=np.int32)
    w_att = np.ascontiguousarray(w_att, dtype=np.float32)

    sched, per_core = host_prep(src, dst)
    if sched not in _cache:
        _cache[sched] = build_program(sched)
    nc = _cache[sched]

    in_maps = make_in_maps(h_features, w_att, per_core)
    res = run_bass_kernel_spmd(nc, in_maps, list(range(ncores)))
    y_list = [res.results[c]["y"] for c in range(ncores)]
    return assemble(y_list, h_features, per_core).astype(np.float32)
